# revision 11
# baseline (speedup 1.0000x reference)
"""Grouped MoE MLP (64 experts) on 8 Trainium2 NeuronCores.

Strategy: expert parallelism. Each core owns 8 experts (size-sorted "snake"
assignment so every core gets the same per-slot padded token capacity and the
padding is tight). Both matmuls keep tokens as the moving operand:

    hT[f, t]   = w1t[e] (stationary, [h,f] tiles) @ xT (moving, [h, t])
    hT         = gelu(hT)                     (ScalarE, PSUM f32 -> SBUF bf16)
    outT[o, t] = w2[e] (stationary, [f,o] tiles) @ hT (moving, [f, t])

All DRAM tensors are laid out host-side so every DMA moves long contiguous
runs per partition (8KB for weights, ~4KB for activations):

    w1n[s, hi, fg, ko, fs] = w1[e_s, fg*512+fs, ko*128+hi]   (bf16)
    w2n[s, fi, og, fo, hs] = w2[e_s, fo*128+fi, og*512+hs]   (bf16)
    xn [hi, slot-block s: ko*Cj + t] = x_s[t, ko*128+hi]     (bf16)
    outn[oi, slot-block s: oo*Cj + t] = out_s[t, oo*128+oi]  (bf16)

Weights stream HBM->SBUF once per core (double-buffered 1MiB chunks), PSUM
accumulates in f32, output is written bf16 and upcast/un-permuted on host.
"""

import numpy as np

NCORES = 8
SLOTS = 8  # experts per core
NE = 64
H = 1024
F = 2048
T = 16384
P = 128
KO = H // P  # 8  k-tiles for mm1 (contraction over H)
FO = F // P  # 16 f-tiles (mm1 output tiles / mm2 contraction)
OO = H // P  # 8  output h-tiles for mm2
FS = 512  # w1 f-chunk width (DMA chunk granularity)
FG = F // FS  # 4 w1 chunks per slot
HS = 512  # w2 h-chunk width
OG = H // HS  # 2 w2 h-groups per slot
NMAX = 512  # max moving-operand length (one fp32 PSUM bank)

ACT_FN = "Gelu"  # overridable for CoreSim tests (Gelu not implemented there)

_prog_cache = {}


def _build_program(C):
    """Build the SPMD Bass program for per-slot token capacities C (len SLOTS)."""
    from contextlib import ExitStack

    import concourse.tile as tile
    from concourse import bacc, mybir
    from concourse.bass import MemorySpace

    bf16 = mybir.dt.bfloat16
    f32 = mybir.dt.float32
    CTOT = int(sum(C))

    nc = bacc.Bacc("TRN2", target_bir_lowering=False, debug=False, num_devices=NCORES)
    w1n_d = nc.dram_tensor("w1n", [SLOTS, P, FG * KO * FS], bf16, kind="ExternalInput").ap()
    w2n_d = nc.dram_tensor("w2n", [SLOTS, P, OG * FO * HS], bf16, kind="ExternalInput").ap()
    xn_d = nc.dram_tensor("xn", [P, KO * CTOT], bf16, kind="ExternalInput").ap()
    outn_d = nc.dram_tensor("outn", [P, OO * CTOT], bf16, kind="ExternalOutput").ap()

    with tile.TileContext(nc) as tc, ExitStack() as ctx:
        w1_pool = ctx.enter_context(tc.tile_pool(name="w1", bufs=5))
        w2_pool = ctx.enter_context(tc.tile_pool(name="w2", bufs=2))
        x_pool = ctx.enter_context(tc.tile_pool(name="x", bufs=2))
        h_pool = ctx.enter_context(tc.tile_pool(name="h", bufs=2))
        o_pool = ctx.enter_context(tc.tile_pool(name="o", bufs=2))
        ph_pool = ctx.enter_context(
            tc.tile_pool(name="ph", bufs=4, space=MemorySpace.PSUM)
        )
        po_pool = ctx.enter_context(
            tc.tile_pool(name="po", bufs=4, space=MemorySpace.PSUM)
        )

        # Weight DMAs go through nc.sync (one FIFO HWDGE queue) in exact
        # consumption order, slot by slot: FIFO completion order == issue
        # order, so the bytes the PE needs next always get the bandwidth.
        # x loads ride the scalar engine's separate HWDGE ring (parallel
        # dispatch at kernel start), output stores ride the gpsimd SWDGE
        # ring, so neither blocks the weight stream. w1 is split into two
        # half-tiles (bufs=5) for an extra half-slot of prefetch lookahead.
        off = 0
        for j in range(SLOTS):
            Cj = int(C[j])
            x_sb = x_pool.tile([P, KO * Cj], bf16, tag="x")
            nc.scalar.dma_start(x_sb, xn_d[:, KO * off : KO * (off + Cj)])
            w1h = KO * FS  # half-tile columns (2 fg chunks)
            w1_sba = w1_pool.tile([P, 2 * w1h], bf16, tag="w1")
            w1_sbb = w1_pool.tile([P, 2 * w1h], bf16, tag="w1")
            for fg in range(FG):  # 1MiB chunks, 8KB contiguous runs per partition
                t = w1_sba if fg < 2 else w1_sbb
                c0 = (fg % 2) * w1h
                nc.sync.dma_start(
                    t[:, c0 : c0 + w1h], w1n_d[j, :, fg * w1h : (fg + 1) * w1h]
                )
            w2_sb = w2_pool.tile([P, OG * FO * HS], bf16, tag="w2")
            fh = FO // 2
            for wc in range(2 * OG):  # 1MiB chunks (og, fo-half)
                c0 = wc * fh * HS
                nc.sync.dma_start(
                    w2_sb[:, c0 : c0 + fh * HS], w2n_d[j, :, c0 : c0 + fh * HS]
                )
            o_sb = o_pool.tile([P, OO * Cj], bf16, tag="o")

            for nb in range(0, Cj, NMAX):
                NB = min(NMAX, Cj - nb)
                h_sb = h_pool.tile([P, FO * NB], bf16, tag="h")
                for fo in range(FO):
                    fg, fs = fo // 4, (fo % 4) * P
                    w1t, fgl = (w1_sba, fg) if fg < 2 else (w1_sbb, fg - 2)
                    ph = ph_pool.tile([P, NMAX], f32, tag="ph")
                    for ko in range(KO):
                        c0 = (fgl * KO + ko) * FS + fs
                        nc.tensor.matmul(
                            ph[:, :NB],
                            w1t[:, c0 : c0 + P],
                            x_sb[:, ko * Cj + nb : ko * Cj + nb + NB],
                            start=(ko == 0),
                            stop=(ko == KO - 1),
                        )
                    nc.scalar.activation(
                        h_sb[:, fo * NB : (fo + 1) * NB],
                        ph[:, :NB],
                        getattr(mybir.ActivationFunctionType, ACT_FN),
                    )
                for oo in range(OO):
                    og, hs = oo // 4, (oo % 4) * P
                    po = po_pool.tile([P, NMAX], f32, tag="po")
                    for fo in range(FO):
                        nc.tensor.matmul(
                            po[:, :NB],
                            w2_sb[:, (og * FO + fo) * HS + hs : (og * FO + fo) * HS + hs + P],
                            h_sb[:, fo * NB : fo * NB + NB],
                            start=(fo == 0),
                            stop=(fo == FO - 1),
                        )
                    nc.vector.tensor_copy(
                        o_sb[:, oo * Cj + nb : oo * Cj + nb + NB], po[:, :NB]
                    )
            nc.gpsimd.dma_start(outn_d[:, OO * off : OO * (off + Cj)], o_sb)
            off += Cj

    nc.compile()
    return nc


def _get_program(C):
    key = tuple(int(c) for c in C)
    if key not in _prog_cache:
        _prog_cache[key] = _build_program(key)
    return _prog_cache[key]


def plan(sizes):
    """Expert->core/slot assignment + slot capacities from token counts."""
    sizes = np.asarray(sizes, np.int64)
    assert sizes.shape == (NE,) and sizes.sum() == T
    order = np.argsort(-sizes, kind="stable")  # descending
    # expert_of[core][slot]
    expert_of = [[int(order[s * NCORES + c]) for s in range(SLOTS)] for c in range(NCORES)]
    C = []
    for s in range(SLOTS):
        m = max(int(sizes[order[s * NCORES + c]]) for c in range(NCORES))
        C.append(max(2, -(-m // 2) * 2))  # round up to multiple of 2, min 2
    offs = np.concatenate([[0], np.cumsum(C)]).astype(np.int64)
    return expert_of, C, offs


def prepare_inputs(x, w1, w2, sizes, expert_of, C, offs):
    """Host-side shard/pad/transpose/cast. Returns per-core input maps."""
    import ml_dtypes

    bf16 = ml_dtypes.bfloat16
    x = np.asarray(x, np.float32)
    tok_offs = np.concatenate([[0], np.cumsum(sizes)]).astype(np.int64)
    w1_bf = np.asarray(w1, np.float32).astype(bf16)  # [NE, F, H]
    w2_bf = np.asarray(w2, np.float32).astype(bf16)  # [NE, F, H]
    CTOT = int(sum(C))

    in_maps = []
    for c in range(NCORES):
        experts = expert_of[c]
        # w1n[s, hi, fg, ko, fs] = w1[e, fg*FS+fs, ko*P+hi]
        w1n = np.ascontiguousarray(
            w1_bf[experts].reshape(SLOTS, FG, FS, KO, P).transpose(0, 4, 1, 3, 2)
        ).reshape(SLOTS, P, FG * KO * FS)
        # w2n[s, fi, og, fo, hs] = w2[e, fo*P+fi, og*HS+hs]
        w2n = np.ascontiguousarray(
            w2_bf[experts].reshape(SLOTS, FO, P, OG, HS).transpose(0, 2, 3, 1, 4)
        ).reshape(SLOTS, P, OG * FO * HS)
        xn = np.zeros((P, KO * CTOT), bf16)
        for s, e in enumerate(experts):
            n = int(sizes[e])
            Cj = int(C[s])
            xs = np.zeros((Cj, KO, P), np.float32)
            xs[:n] = x[tok_offs[e] : tok_offs[e] + n].reshape(n, KO, P)
            # xn block: [P, KO, Cj]
            xn[:, KO * offs[s] : KO * (offs[s] + Cj)] = (
                xs.transpose(2, 1, 0).reshape(P, KO * Cj).astype(bf16)
            )
        in_maps.append({"w1n": w1n, "w2n": w2n, "xn": xn})
    return in_maps


def scatter_output(results, sizes, expert_of, C, offs):
    """Gather per-core outputs back into the full [T, H] f32 output."""
    tok_offs = np.concatenate([[0], np.cumsum(sizes)]).astype(np.int64)
    out = np.empty((T, H), np.float32)
    for c in range(NCORES):
        outn = np.asarray(results[c]["outn"])  # [P, OO*CTOT] bf16
        for s, e in enumerate(expert_of[c]):
            n = int(sizes[e])
            Cj = int(C[s])
            blk = outn[:, OO * offs[s] : OO * (offs[s] + Cj)].reshape(P, OO, Cj)
            # out[t, oo*P+oi] = blk[oi, oo, t]
            out[tok_offs[e] : tok_offs[e] + n] = (
                blk[:, :, :n].transpose(2, 1, 0).reshape(n, H).astype(np.float32)
            )
    return out


def kernel(x, w1, w2, tokens_per_expert):
    from concourse import bass2jax

    sizes = np.asarray(tokens_per_expert, np.int64)
    expert_of, C, offs = plan(sizes)
    nc = _get_program(C)
    in_maps = prepare_inputs(x, w1, w2, sizes, expert_of, C, offs)
    results = bass2jax.run_bass_via_pjrt(nc, in_maps, n_cores=NCORES)
    return scatter_output(results, sizes, expert_of, C, offs)


# revision 12
# speedup vs baseline: 1.1484x; 1.1484x over previous
"""Grouped MoE MLP (64 experts) on 8 Trainium2 NeuronCores.

Strategy: expert parallelism. Each core owns 8 experts (size-sorted "snake"
assignment so every core gets the same per-slot padded token capacity and the
padding is tight). Both matmuls keep tokens as the moving operand:

    hT[f, t]   = w1t[e] (stationary, [h,f] tiles) @ xT (moving, [h, t])
    hT         = gelu(hT)                     (ScalarE, PSUM f32 -> SBUF bf16)
    outT[o, t] = w2[e] (stationary, [f,o] tiles) @ hT (moving, [f, t])

All DRAM tensors are laid out host-side so every DMA moves long contiguous
runs per partition (8KB for weights, ~4KB for activations):

    w1n[s, hi, fg, ko, fs] = w1[e_s, fg*512+fs, ko*128+hi]   (bf16)
    w2n[s, fi, og, fo, hs] = w2[e_s, fo*128+fi, og*512+hs]   (bf16)
    xn [hi, slot-block s: ko*Cj + t] = x_s[t, ko*128+hi]     (bf16)
    outn[oi, slot-block s: oo*Cj + t] = out_s[t, oo*128+oi]  (bf16)

Weights stream HBM->SBUF once per core (double-buffered 1MiB chunks), PSUM
accumulates in f32, output is written bf16 and upcast/un-permuted on host.
"""

import numpy as np

NCORES = 8
SLOTS = 8  # experts per core
NE = 64
H = 1024
F = 2048
T = 16384
P = 128
KO = H // P  # 8  k-tiles for mm1 (contraction over H)
FO = F // P  # 16 f-tiles (mm1 output tiles / mm2 contraction)
OO = H // P  # 8  output h-tiles for mm2
FS = 512  # w1 f-chunk width (DMA chunk granularity)
FG = F // FS  # 4 w1 chunks per slot
HS = 512  # w2 h-chunk width
OG = H // HS  # 2 w2 h-groups per slot
NMAX = 512  # max moving-operand length (one fp32 PSUM bank)

ACT_FN = "Gelu"  # overridable for CoreSim tests (Gelu not implemented there)

_prog_cache = {}


def _build_program(C):
    """Build the SPMD Bass program for per-slot token capacities C (len SLOTS)."""
    from contextlib import ExitStack

    import concourse.tile as tile
    from concourse import bacc, mybir
    from concourse.bass import MemorySpace

    bf16 = mybir.dt.bfloat16
    f32 = mybir.dt.float32
    CTOT = int(sum(C))

    nc = bacc.Bacc("TRN2", target_bir_lowering=False, debug=False, num_devices=NCORES)
    w1n_d = nc.dram_tensor("w1n", [SLOTS, P, FG * KO * FS], bf16, kind="ExternalInput").ap()
    w2n_d = nc.dram_tensor("w2n", [SLOTS, P, OG * FO * HS], bf16, kind="ExternalInput").ap()
    xn_d = nc.dram_tensor("xn", [P, KO * CTOT], bf16, kind="ExternalInput").ap()
    outn_d = nc.dram_tensor("outn", [P, OO * CTOT], bf16, kind="ExternalOutput").ap()

    with tile.TileContext(nc) as tc, ExitStack() as ctx:
        w1_pool = ctx.enter_context(tc.tile_pool(name="w1", bufs=5))
        w2_pool = ctx.enter_context(tc.tile_pool(name="w2", bufs=2))
        x_pool = ctx.enter_context(tc.tile_pool(name="x", bufs=2))
        h_pool = ctx.enter_context(tc.tile_pool(name="h", bufs=2))
        o_pool = ctx.enter_context(tc.tile_pool(name="o", bufs=2))
        ph_pool = ctx.enter_context(
            tc.tile_pool(name="ph", bufs=4, space=MemorySpace.PSUM)
        )
        po_pool = ctx.enter_context(
            tc.tile_pool(name="po", bufs=4, space=MemorySpace.PSUM)
        )

        # Weight DMAs go through nc.sync (one FIFO HWDGE queue) in exact
        # consumption order, slot by slot: FIFO completion order == issue
        # order, so the bytes the PE needs next always get the bandwidth.
        # x loads ride the scalar engine's separate HWDGE ring (parallel
        # dispatch at kernel start), output stores ride the gpsimd SWDGE
        # ring, so neither blocks the weight stream. w1 is split into two
        # half-tiles (bufs=5) for an extra half-slot of prefetch lookahead.
        off = 0
        for j in range(SLOTS):
            Cj = int(C[j])
            x_sb = x_pool.tile([P, KO * Cj], bf16, tag="x")
            nc.scalar.dma_start(x_sb, xn_d[:, KO * off : KO * (off + Cj)])
            w1h = KO * FS  # half-tile columns (2 fg chunks)
            w1_sba = w1_pool.tile([P, 2 * w1h], bf16, tag="w1")
            w1_sbb = w1_pool.tile([P, 2 * w1h], bf16, tag="w1")
            for fg in range(FG):  # 1MiB chunks, 8KB contiguous runs per partition
                t = w1_sba if fg < 2 else w1_sbb
                c0 = (fg % 2) * w1h
                nc.sync.dma_start(
                    t[:, c0 : c0 + w1h], w1n_d[j, :, fg * w1h : (fg + 1) * w1h]
                )
            w2_sb = w2_pool.tile([P, OG * FO * HS], bf16, tag="w2")
            fh = FO // 2
            for wc in range(2 * OG):  # 1MiB chunks (og, fo-half)
                c0 = wc * fh * HS
                nc.sync.dma_start(
                    w2_sb[:, c0 : c0 + fh * HS], w2n_d[j, :, c0 : c0 + fh * HS]
                )
            o_sb = o_pool.tile([P, OO * Cj], bf16, tag="o")

            for nb in range(0, Cj, NMAX):
                NB = min(NMAX, Cj - nb)
                h_sb = h_pool.tile([P, FO * NB], bf16, tag="h")
                for fo in range(FO):
                    fg, fs = fo // 4, (fo % 4) * P
                    w1t, fgl = (w1_sba, fg) if fg < 2 else (w1_sbb, fg - 2)
                    ph = ph_pool.tile([P, NMAX], f32, tag="ph")
                    for ko in range(KO):
                        c0 = (fgl * KO + ko) * FS + fs
                        nc.tensor.matmul(
                            ph[:, :NB],
                            w1t[:, c0 : c0 + P],
                            x_sb[:, ko * Cj + nb : ko * Cj + nb + NB],
                            start=(ko == 0),
                            stop=(ko == KO - 1),
                        )
                    nc.scalar.activation(
                        h_sb[:, fo * NB : (fo + 1) * NB],
                        ph[:, :NB],
                        getattr(mybir.ActivationFunctionType, ACT_FN),
                    )
                for oo in range(OO):
                    og, hs = oo // 4, (oo % 4) * P
                    po = po_pool.tile([P, NMAX], f32, tag="po")
                    for fo in range(FO):
                        nc.tensor.matmul(
                            po[:, :NB],
                            w2_sb[:, (og * FO + fo) * HS + hs : (og * FO + fo) * HS + hs + P],
                            h_sb[:, fo * NB : fo * NB + NB],
                            start=(fo == 0),
                            stop=(fo == FO - 1),
                        )
                    nc.vector.tensor_copy(
                        o_sb[:, oo * Cj + nb : oo * Cj + nb + NB], po[:, :NB]
                    )
            nc.gpsimd.dma_start(outn_d[:, OO * off : OO * (off + Cj)], o_sb)
            off += Cj

    nc.compile()
    return nc


def _get_program(C):
    key = tuple(int(c) for c in C)
    if key not in _prog_cache:
        _prog_cache[key] = _build_program(key)
    return _prog_cache[key]


def plan(sizes):
    """Expert->core/slot assignment + slot capacities from token counts."""
    sizes = np.asarray(sizes, np.int64)
    assert sizes.shape == (NE,) and sizes.sum() == T
    order = np.argsort(-sizes, kind="stable")  # descending
    # expert_of[core][slot]
    expert_of = [[int(order[s * NCORES + c]) for s in range(SLOTS)] for c in range(NCORES)]
    C = []
    for s in range(SLOTS):
        m = max(int(sizes[order[s * NCORES + c]]) for c in range(NCORES))
        # multiple-of-8 keeps every SBUF column slice 16B-aligned (bf16);
        # finer rounding (tried 2) degrades the PE moving-operand stream.
        C.append(max(8, -(-m // 8) * 8))
    offs = np.concatenate([[0], np.cumsum(C)]).astype(np.int64)
    return expert_of, C, offs


def prepare_inputs(x, w1, w2, sizes, expert_of, C, offs):
    """Host-side shard/pad/transpose/cast. Returns per-core input maps."""
    import ml_dtypes

    bf16 = ml_dtypes.bfloat16
    x = np.asarray(x, np.float32)
    tok_offs = np.concatenate([[0], np.cumsum(sizes)]).astype(np.int64)
    w1_bf = np.asarray(w1, np.float32).astype(bf16)  # [NE, F, H]
    w2_bf = np.asarray(w2, np.float32).astype(bf16)  # [NE, F, H]
    CTOT = int(sum(C))

    in_maps = []
    for c in range(NCORES):
        experts = expert_of[c]
        # w1n[s, hi, fg, ko, fs] = w1[e, fg*FS+fs, ko*P+hi]
        w1n = np.ascontiguousarray(
            w1_bf[experts].reshape(SLOTS, FG, FS, KO, P).transpose(0, 4, 1, 3, 2)
        ).reshape(SLOTS, P, FG * KO * FS)
        # w2n[s, fi, og, fo, hs] = w2[e, fo*P+fi, og*HS+hs]
        w2n = np.ascontiguousarray(
            w2_bf[experts].reshape(SLOTS, FO, P, OG, HS).transpose(0, 2, 3, 1, 4)
        ).reshape(SLOTS, P, OG * FO * HS)
        xn = np.zeros((P, KO * CTOT), bf16)
        for s, e in enumerate(experts):
            n = int(sizes[e])
            Cj = int(C[s])
            xs = np.zeros((Cj, KO, P), np.float32)
            xs[:n] = x[tok_offs[e] : tok_offs[e] + n].reshape(n, KO, P)
            # xn block: [P, KO, Cj]
            xn[:, KO * offs[s] : KO * (offs[s] + Cj)] = (
                xs.transpose(2, 1, 0).reshape(P, KO * Cj).astype(bf16)
            )
        in_maps.append({"w1n": w1n, "w2n": w2n, "xn": xn})
    return in_maps


def scatter_output(results, sizes, expert_of, C, offs):
    """Gather per-core outputs back into the full [T, H] f32 output."""
    tok_offs = np.concatenate([[0], np.cumsum(sizes)]).astype(np.int64)
    out = np.empty((T, H), np.float32)
    for c in range(NCORES):
        outn = np.asarray(results[c]["outn"])  # [P, OO*CTOT] bf16
        for s, e in enumerate(expert_of[c]):
            n = int(sizes[e])
            Cj = int(C[s])
            blk = outn[:, OO * offs[s] : OO * (offs[s] + Cj)].reshape(P, OO, Cj)
            # out[t, oo*P+oi] = blk[oi, oo, t]
            out[tok_offs[e] : tok_offs[e] + n] = (
                blk[:, :, :n].transpose(2, 1, 0).reshape(n, H).astype(np.float32)
            )
    return out


def kernel(x, w1, w2, tokens_per_expert):
    from concourse import bass2jax

    sizes = np.asarray(tokens_per_expert, np.int64)
    expert_of, C, offs = plan(sizes)
    nc = _get_program(C)
    in_maps = prepare_inputs(x, w1, w2, sizes, expert_of, C, offs)
    results = bass2jax.run_bass_via_pjrt(nc, in_maps, n_cores=NCORES)
    return scatter_output(results, sizes, expert_of, C, offs)
